# revision 12
# baseline (speedup 1.0000x reference)
"""Trainium2 Bass kernel for nn_CrossAttention_31791347925417.

Math (per batch b, per stream tok in {x, blood} with weight W in {W1, W2}):
    kv = tok @ W.T ; k, v heads [H, N, D]
    ctx = softmax_d( SCALE * k_h^T v_h )          # [H, D, D], softmax over first D
    out_x = x_h @ ctx2_h ; out_b = blood_h @ ctx1_h

Refactor used here (Gram trick):
    k_h^T v_h = W_k_h (tok^T tok) W_v_h^T  with G = tok^T tok  [C, C]
so the N=4096 contraction happens once (G) instead of twice (k and v), and
everything downstream is tiny [C,C]-scale work.

We compute ctxT_h = (SCALE*W_k applied) via  Q = G @ WkT, then per head-pair
a full [128,128] product  WvT_pair^T @ Q_pair  whose DIAGONAL 64x64 blocks are
ctxT_h [e, d] for the two heads (off-diagonal blocks are unused).  Softmax runs
along the free axis (d).  The normalized probs are written into the diagonal
blocks of a zeroed [128,128] tile F; BD = F^T (PE transpose) is the
block-diagonal ctx pair used by the output matmuls:
    out[n, (h,e)] = sum_{(h,d)} xT[(h,d), n] * BD[(h,d), (h,e)]

Sharding: data-parallel over batch B=8 across the 8 cores; weights replicated.
Host pre-transposes W -> W.T [C, 2C] and folds SCALE into the k-half (exact,
SCALE = 0.125).
"""

import sys

if "/opt/trn_rl_repo" not in sys.path:
    sys.path.insert(0, "/opt/trn_rl_repo")

import numpy as np

from concourse import bacc, masks, mybir, tile
from concourse.bass_utils import run_bass_kernel_spmd

B, N, C, H = 8, 4096, 512, 8
D = C // H
SCALE = D ** -0.5
P = 128
NBIG = N // 512          # 8 big row tiles (512 rows each)
NT = N // P              # 32 n-tiles
CB = C // P              # 4 column blocks == head pairs
F32 = mybir.dt.float32
F32R = mybir.dt.float32r
BF16 = mybir.dt.bfloat16
AX = mybir.AxisListType
ACT_EXP = mybir.ActivationFunctionType.Exp

# precision knobs
G_F32R = True      # G = tok^T tok in float32r (4x faster than float32)
Q_F32R = True      # Q = G @ WkT in float32r
OUT_BF16 = True    # final out matmuls in bf16 (vs float32)
TRANS_BF16_ID = False  # walrus rejects mixed f32r/bf16 matmul operands
TRANS_F32R = False  # walrus codegen rejects f32r transpose-mode


def _r(ap):
    return ap.bitcast(F32R)


def build_nc():
    nc = bacc.Bacc("TRN2", target_bir_lowering=False, debug=False)

    TOKDT = F32R if G_F32R else F32
    WDT = F32R if Q_F32R else F32
    xb = nc.dram_tensor("xb", [N, C], TOKDT, kind="ExternalInput").ap()
    bb = nc.dram_tensor("bb", [N, C], TOKDT, kind="ExternalInput").ap()
    w1t = nc.dram_tensor("w1t", [C, 2 * C], WDT, kind="ExternalInput").ap()
    w2t = nc.dram_tensor("w2t", [C, 2 * C], WDT, kind="ExternalInput").ap()
    ox = nc.dram_tensor("ox", [N, C], F32, kind="ExternalOutput").ap()
    ob = nc.dram_tensor("ob", [N, C], F32, kind="ExternalOutput").ap()

    with tile.TileContext(nc) as tc:
        _emit(nc, tc, xb, bb, w1t, w2t, ox, ob)

    nc.compile()
    return nc


def _emit(nc, tc, xb, bb, w1t, w2t, ox, ob):
    TOKDT = F32R if G_F32R else F32
    WDT = F32R if Q_F32R else F32
    from contextlib import ExitStack

    ctx = ExitStack()
    with ctx:
        const = ctx.enter_context(tc.tile_pool(name="const", bufs=1))
        wpool = ctx.enter_context(tc.tile_pool(name="wpool", bufs=1))
        tokp = ctx.enter_context(tc.tile_pool(name="tokp", bufs=9))
        xtp = ctx.enter_context(tc.tile_pool(name="xtp", bufs=1))
        gqp = ctx.enter_context(tc.tile_pool(name="gqp", bufs=8))
        smallp = ctx.enter_context(tc.tile_pool(name="smallp", bufs=2))
        fpool = ctx.enter_context(tc.tile_pool(name="fpool", bufs=2))
        bdpool = ctx.enter_context(tc.tile_pool(name="bdpool", bufs=8))
        ostp = ctx.enter_context(tc.tile_pool(name="ostp", bufs=2))
        psG = ctx.enter_context(tc.tile_pool(name="psG", bufs=4, space="PSUM"))
        psT = ctx.enter_context(tc.tile_pool(name="psT", bufs=2, space="PSUM"))
        psO = ctx.enter_context(tc.tile_pool(name="psO", bufs=2, space="PSUM"))

        ident = const.tile([P, P], F32, tag="idf")
        masks.make_identity(nc, ident[:])
        ident_bf = const.tile([P, P], BF16, tag="idb")
        masks.make_identity(nc, ident_bf[:])
        if TRANS_F32R:
            ident_r = const.tile([P, P], F32R, tag="idr")
            masks.make_identity(nc, ident_r[:])
        else:
            ident_r = None

        # weights: chunk j (c-rows 128j..128j+128) lives at cols [j*2C, (j+1)*2C)
        w_x = wpool.tile([P, CB * 2 * C], WDT, tag="wx")
        w_b = wpool.tile([P, CB * 2 * C], WDT, tag="wb")

        def load_weights():
            nc.sync.dma_start(
                w_x[:].rearrange("p (j c) -> p j c", j=CB),
                w1t[:, :].rearrange("(j p) c -> p j c", p=P),
            )
            nc.sync.dma_start(
                w_b[:].rearrange("p (j c) -> p j c", j=CB),
                w2t[:, :].rearrange("(j p) c -> p j c", p=P),
            )

        def wchunk(w, j):
            return w[:, j * 2 * C:(j + 1) * 2 * C]

        # transposed tokens, bf16: pair block m at cols [m*N, (m+1)*N)
        xT_x = xtp.tile([P, CB * N], BF16, tag="xtx")
        xT_b = xtp.tile([P, CB * N], BF16, tag="xtb")

        def out_tile(ops_pool_tag, xT, BDs, k):
            """4 pair matmuls for n-tile k -> one [P, C] psum tile."""
            ops = psO.tile([P, C], F32, tag="o")
            for p in range(CB):
                lhsT = xT[:, k * C + p * P:k * C + (p + 1) * P]
                rhs = BDs[p][:]
                if not OUT_BF16:
                    lhsT = lhsT.bitcast(F32)
                    rhs = rhs.bitcast(F32)
                nc.tensor.matmul(
                    ops[:, p * P:(p + 1) * P], lhsT, rhs, start=True, stop=True
                )
            return ops

        def stream_context(tok_dram, w, xT, inline=None):
            """Load tok; dense G matmul phase (keeps PE HAM-warm); transpose
            phase; then Q/ctx/softmax -> BD tiles.

            inline=(BDs, out_dram): during the transpose phase, compute each
            n-tile of the output right after its transpose lands in xT.
            """
            toks = []
            for kb in range(NBIG):
                tokb = tokp.tile([P, 4 * C], TOKDT, tag="tok", name=f"tok{kb}")
                nc.sync.dma_start(
                    tokb[:].rearrange("p (s c) -> p s c", s=4),
                    tok_dram[kb * 512:(kb + 1) * 512, :].rearrange(
                        "(s p) c -> p s c", p=P
                    ),
                )
                toks.append(tokb)

            # dense G phase: 128 back-to-back matmuls
            gps = [psG.tile([P, C], F32, tag="g", name=f"gps{m}") for m in range(CB)]
            for kb in range(NBIG):
                for sub in range(4):
                    k = kb * 4 + sub
                    sb = toks[kb][:, sub * C:(sub + 1) * C]
                    for m in range(CB):
                        nc.tensor.matmul(
                            gps[m][:], sb[:, m * P:(m + 1) * P], sb,
                            start=(k == 0), stop=(k == NT - 1),
                        )

            # transpose phase (+ inline outputs for the cross stream)
            for kb in range(NBIG):
                ost = ostp.tile([P, 4 * C], F32, tag="ost", name="ost") if inline else None
                for sub in range(4):
                    k = kb * 4 + sub
                    sb = toks[kb][:, sub * C:(sub + 1) * C]
                    tps = psT.tile([P, C], TOKDT if TRANS_F32R else F32, tag="t")
                    for m in range(CB):
                        nc.tensor.transpose(
                            tps[:, m * P:(m + 1) * P],
                            sb[:, m * P:(m + 1) * P] if TRANS_F32R
                            else sb[:, m * P:(m + 1) * P].bitcast(F32),
                            ident_r[:] if TRANS_F32R else ident[:],
                        )
                    # drain transposes into xT (cast to bf16); k-major layout
                    nc.vector.tensor_copy(xT[:, k * C:(k + 1) * C], tps[:])
                    if inline:
                        BDs, out_dram = inline
                        ops = out_tile("o", xT, BDs, k)
                        nc.vector.tensor_copy(ost[:, sub * C:(sub + 1) * C], ops[:])
                if inline:
                    _, out_dram = inline
                    nc.sync.dma_start(
                        out_dram[kb * 512:(kb + 1) * 512, :].rearrange(
                            "(s p) c -> p s c", p=P
                        ),
                        ost[:].rearrange("p (s c) -> p s c", s=4),
                    )

            # drain G
            g_sb = []
            for m in range(CB):
                g = gqp.tile([P, C], WDT, tag="gq")
                nc.scalar.copy(g[:], gps[m][:])
                g_sb.append(g)

            # Q = G @ WkT_scaled  [C, C]  (SCALE folded into WkT on host)
            q_sb = []
            for i in range(CB):
                qp = psG.tile([P, C], F32, tag="g")
                for j in range(CB):
                    nc.tensor.matmul(
                        qp[:], g_sb[j][:, i * P:(i + 1) * P],
                        wchunk(w, j)[:, 0:C], start=(j == 0), stop=(j == 3),
                    )
                q = gqp.tile([P, C], WDT, tag="gq")
                nc.scalar.copy(q[:], qp[:])
                q_sb.append(q)

            # per head-pair p: full [128,128] = WvT_pair^T @ Q_pair; diagonal
            # 64x64 blocks are ctxT_h [e, d] for heads 2p, 2p+1
            BDs = []
            for p in range(CB):
                cps = psO.tile([P, P], F32, tag="o")
                for j in range(CB):
                    nc.tensor.matmul(
                        cps[:],
                        wchunk(w, j)[:, C + p * P:C + (p + 1) * P],
                        q_sb[j][:, p * P:(p + 1) * P],
                        start=(j == 0),
                        stop=(j == 3),
                    )
                # softmax along free axis (d) on the two diagonal blocks
                nm = smallp.tile([P, 1], F32, tag="nm")
                sm = smallp.tile([P, 1], F32, tag="sm")
                rv = smallp.tile([P, 1], F32, tag="rv")
                pp = smallp.tile([P, D], F32, tag="pp")
                fp = fpool.tile([P, P], BF16, tag="F")
                nc.gpsimd.memset(fp[:], 0.0)
                for d in range(2):
                    s0 = slice(d * D, (d + 1) * D)
                    blk = cps[s0, s0]
                    nc.vector.reduce_max(nm[s0, :], blk, axis=AX.X, negate=True)
                    nc.scalar.activation(
                        pp[s0, :], blk, ACT_EXP, bias=nm[s0, :], scale=1.0,
                        accum_out=sm[s0, :],
                    )
                nc.vector.reciprocal(rv[:], sm[:])
                for d in range(2):
                    s0 = slice(d * D, (d + 1) * D)
                    nc.vector.tensor_scalar_mul(fp[s0, s0], pp[s0, :], rv[s0, :])
                # BD = F^T : block-diag(ctx_2p, ctx_2p+1) with (h,d) rows
                bps = psT.tile([P, P], BF16, tag="t")
                nc.tensor.transpose(bps[:, 0:P], fp[:], ident_bf[:])
                bd = bdpool.tile([P, P], BF16, tag="bd")
                nc.vector.tensor_copy(bd[:], bps[:, 0:P])
                BDs.append(bd)
            return BDs

        # phase A: stream x -> ctx1 (keep xT_x for phase C)
        load_weights()
        bd1 = stream_context(xb, w_x, xT_x)
        # phase B: stream blood -> ctx2; out_b = blood @ ctx1 inline
        bd2 = stream_context(bb, w_b, xT_b, inline=(bd1, ob))
        # phase C: out_x = x @ ctx2
        for kb in range(NBIG):
            ost = ostp.tile([P, 4 * C], F32, tag="ost")
            for sub in range(4):
                k = kb * 4 + sub
                ops = out_tile("o", xT_x, bd2, k)
                nc.vector.tensor_copy(ost[:, sub * C:(sub + 1) * C], ops[:])
            nc.sync.dma_start(
                ox[kb * 512:(kb + 1) * 512, :].rearrange("(s p) c -> p s c", p=P),
                ost[:].rearrange("p (s c) -> p s c", s=4),
            )


_NC_CACHE = None


def _get_nc():
    global _NC_CACHE
    if _NC_CACHE is None:
        _NC_CACHE = build_nc()
    return _NC_CACHE


def _prep_inputs(x, blood, W1, W2):
    x = np.ascontiguousarray(np.asarray(x, dtype=np.float32))
    blood = np.ascontiguousarray(np.asarray(blood, dtype=np.float32))
    w1t = np.ascontiguousarray(np.asarray(W1, dtype=np.float32).T)
    w2t = np.ascontiguousarray(np.asarray(W2, dtype=np.float32).T)
    w1t[:, :C] *= SCALE  # fold softmax scale into the k-projection (exact: 2^-3)
    w2t[:, :C] *= SCALE
    return [
        {"xb": x[b], "bb": blood[b], "w1t": w1t, "w2t": w2t} for b in range(B)
    ]


def kernel(x, blood, W1, W2, trace=False):
    nc = _get_nc()
    in_maps = _prep_inputs(x, blood, W1, W2)
    res = run_bass_kernel_spmd(nc, in_maps, core_ids=list(range(B)), trace=trace)
    out_x = np.stack([res.results[b]["ox"] for b in range(B)])
    out_b = np.stack([res.results[b]["ob"] for b in range(B)])
    if trace:
        kernel.last_results = res
    return (out_x, out_b)
